# revision 25
# baseline (speedup 1.0000x reference)
"""Kabsch loss kernel for Trainium2 (8 NeuronCores, data-parallel over batch).

Math: for each batch b (128 points, 3 dims):
  loss_b = ||xc||_F^2 + ||yc||_F^2 - 2 * nuclear_norm(C),  C = xc^T yc (3x3)
because R = U Vh from SVD(C) gives tr(R^T C) = sum of singular values.
nuclear_norm(C) is computed from the invariants of C (I1=||C||_F^2,
I2 = 2nd invariant of C^T C, e3=|det C|) via Newton iteration on the quartic
  n^4 - 2*I1*n^2 - 8*e3*n + (I1^2 - 4*I2) = 0   (largest root = sigma1+sigma2+sigma3)
normalized so I1 -> 3.

Dataflow per super-tile (512 clouds), software-pipelined so every consumer
runs one iteration behind its producer (in-order engine queues never stall on
same-iteration data):
  x: SWDGE cast-load HBM f32 -> SBUF bf16;  y: HWDGE f32 (SP) + SWDGE SBUF cast
  PE: 24 transposes -> point-major planes in one [128,3072] PSUM tile (x|y)
  ACT/DVE: evacuate planes to SBUF xT/yT (each copy has a single PSUM input --
    hardware allows at most one PSUM operand, and GPSIMD cannot touch PSUM)
  DVE/Pool: 3 triple-wide products x_j (x) [y0|y1|y2] -> [128,9,512] bf16
  PE: 15-matmul ones-chain (sel trick) -> per-cloud G/sy/sx rows in PSUM
  ACT: ssq-x accumulator; DVE: ssq-y accumulator
  PE: 4 [15,128]->[128,15] stat transposes;  ACT: strided copy to q-major grid
Final math (Newton quartic, 2 iters) runs as two [128,32] column halves: the
first is pumped a few ops per iteration into engine idle slots mid-loop, the
second runs at the tail with its det/M chains routed to the then-idle Pool.
"""

import sys

sys.path.insert(0, "/opt/trn_rl_repo")

from contextlib import ExitStack

import numpy as np
import ml_dtypes

import concourse.bass as bass
import concourse.tile as tile
from concourse import bacc, mybir
from concourse.bass_utils import run_bass_kernel_spmd

DT = mybir.dt
ALU = mybir.AluOpType
ACT = mybir.ActivationFunctionType

N_CORES = 8
B_TOTAL = 65536
N_PTS = 128
B_CORE = B_TOTAL // N_CORES  # 8192
F = N_PTS * 3  # 384


def _bv(base_ap, dims, extra_offset=0):
    """Build an AP reusing base_ap's partition dim + offset with custom free dims."""
    return bass.AP(base_ap.tensor, base_ap.offset + extra_offset,
                   [list(base_ap.ap[0])] + [list(d) for d in dims])


def build_kernel(b_core=B_CORE, n_cores=N_CORES):
    n_tiles = b_core // 128
    assert n_tiles % 4 == 0
    n_supers = n_tiles // 4
    W = n_tiles  # loss columns (single final pass)

    nc = bacc.Bacc("TRN2", target_bir_lowering=False, debug=False, num_devices=n_cores)
    x_d = nc.dram_tensor("x", [b_core, F], DT.float32, kind="ExternalInput").ap()
    y_d = nc.dram_tensor("y", [b_core, F], DT.float32, kind="ExternalInput").ap()
    sel_d = nc.dram_tensor("sel", [128, 128], DT.bfloat16, kind="ExternalInput").ap()
    idb_d = nc.dram_tensor("idb", [128, 128], DT.bfloat16, kind="ExternalInput").ap()
    idf_d = nc.dram_tensor("idf", [128, 128], DT.float32, kind="ExternalInput").ap()
    loss_d = nc.dram_tensor("loss", [128, W], DT.float32, kind="ExternalOutput").ap()
    ssq_d = nc.dram_tensor("ssq", [128, 2 * n_supers], DT.float32, kind="ExternalOutput").ap()

    with tile.TileContext(nc) as tc:
        with ExitStack() as ctx:
            _kabsch(ctx, tc, x_d, y_d, sel_d, idb_d, idf_d, loss_d, ssq_d,
                    n_tiles, n_supers, W)
    nc.compile()
    return nc


def _kabsch(ctx, tc, x_d, y_d, sel_d, idb_d, idf_d, loss_d, ssq_d,
            n_tiles, n_supers, W):
    nc = tc.nc
    singles = ctx.enter_context(tc.tile_pool(name="singles", bufs=1))
    loads = ctx.enter_context(tc.tile_pool(name="loads", bufs=5))
    casts = ctx.enter_context(tc.tile_pool(name="casts", bufs=5))
    prods = ctx.enter_context(tc.tile_pool(name="prods", bufs=4))
    stp = ctx.enter_context(tc.tile_pool(name="stp", bufs=4))
    xtp = ctx.enter_context(tc.tile_pool(name="xtp", bufs=3))
    ytp = ctx.enter_context(tc.tile_pool(name="ytp", bufs=3))
    junkp = ctx.enter_context(tc.tile_pool(name="junkp", bufs=3))
    fin = ctx.enter_context(tc.tile_pool(name="fin", bufs=1))
    psum = ctx.enter_context(tc.tile_pool(name="psum", bufs=2, space="PSUM"))

    # constants
    sel = singles.tile([128, 128], DT.bfloat16, tag="sel")
    idb = singles.tile([128, 128], DT.bfloat16, tag="idb")
    idf = singles.tile([128, 128], DT.float32, tag="idf")
    nc.sync.dma_start(out=sel, in_=sel_d)
    nc.sync.dma_start(out=idb, in_=idb_d)
    nc.sync.dma_start(out=idf, in_=idf_d)
    ones = sel[:, 127:128]  # [128,1] bf16 ones column

    # persistent accumulators
    ssq_cols = singles.tile([128, 2 * n_supers], DT.float32, tag="ssq_cols", name="ssq_cols")
    stats = singles.tile([128, 15 * W], DT.float32, tag="stats", name="stats")
    loss = singles.tile([128, W], DT.float32, tag="loss", name="loss")

    def emit_loads(s, y_eng=None):
        xb = loads.tile([128, 4 * F], DT.bfloat16, tag="xb", name=f"xb{s}")
        yf = loads.tile([128, 4 * F], DT.float32, tag="yf", name=f"yf{s}")
        nc.gpsimd.dma_start(
            out=xb[:].rearrange("p (t f) -> p t f", t=4),
            in_=x_d[512 * s:512 * (s + 1), :].rearrange("(t p) f -> p t f", p=128))
        (y_eng or nc.sync).dma_start(
            out=yf[:].rearrange("p (t f) -> p t f", t=4),
            in_=y_d[512 * s:512 * (s + 1), :].rearrange("(t p) f -> p t f", p=128))
        return xb, yf

    def emit_cast(s, yf):
        yb = casts.tile([128, 4 * F], DT.bfloat16, tag="yb", name=f"yb{s}")
        nc.gpsimd.dma_start(out=yb, in_=yf)  # SBUF->SBUF cast
        return yb

    def emit_trans(s, xb, yb):
        """transposes (PE) + ssq (ACT) for super s. pT layout: x0|x1|x2|y0|y1|y2."""
        jx = junkp.tile([128, 4 * F], DT.bfloat16, tag="jx")
        jy = junkp.tile([128, 4 * F], DT.bfloat16, tag="jy")
        pT = psum.tile([128, 3072], DT.bfloat16, tag="pT", name=f"pT{s}")
        for t in range(4):
            for j in range(3):
                nc.tensor.transpose(
                    out=pT[:, 512 * j + 128 * t:512 * j + 128 * (t + 1)],
                    in_=_bv(xb[:], [[3, 128]], extra_offset=F * t + j), identity=idb)
                nc.tensor.transpose(
                    out=pT[:, 1536 + 512 * j + 128 * t:1536 + 512 * j + 128 * (t + 1)],
                    in_=_bv(yb[:], [[3, 128]], extra_offset=F * t + j), identity=idb)
        nc.scalar.activation(out=jx, in_=xb, func=ACT.Square,
                             accum_out=ssq_cols[:, s:s + 1])
        nc.vector.scalar_tensor_tensor(
            out=jy, in0=yb, scalar=1.0, in1=yb, op0=ALU.mult, op1=ALU.mult,
            accum_out=ssq_cols[:, n_supers + s:n_supers + s + 1])
        return pT

    def emit_pp(s, pT):
        """plane evacuation (ACT/DVE, one-PSUM-input copies) + products (DVE)."""
        xT = xtp.tile([128, 1536], DT.bfloat16, tag="xT", name=f"xT{s}")
        yT = ytp.tile([128, 1536], DT.bfloat16, tag="yT", name=f"yT{s}")
        nc.scalar.copy(out=xT, in_=pT[:, 0:1536])
        nc.vector.tensor_copy(out=yT[:, 0:1024], in_=pT[:, 1536:2560])
        nc.scalar.copy(out=yT[:, 1024:1536], in_=pT[:, 2560:3072])
        prod = prods.tile([128, 9, 512], DT.bfloat16, tag="prod", name=f"prod{s}")
        xTap = xT[:]
        for j in range(2, -1, -1):
            xj_rep = _bv(xTap, [[0, 3], [1, 512]], extra_offset=512 * j)
            on_pool = j == 2 or (j == 1 and s % 2 == 1)
            eng = nc.gpsimd if on_pool else nc.vector
            eng.tensor_mul(
                prod[:, 3 * j:3 * j + 3, :].rearrange("p a c -> p (a c)"),
                xj_rep, yT)
        return prod, xT, yT

    def emit_stats(s, prod, xT, yT):
        """15-matmul chain into pstat rows 0..14 (PE), ACT evac to st."""
        pstat = psum.tile([128, 512], DT.float32, tag="pstat", name=f"pstat{s}")
        for q in range(14, -1, -1):
            if q < 9:
                rhs = prod[:, q, :]
            elif q < 12:
                rhs = yT[:, 512 * (q - 9):512 * (q - 8)]
            else:
                rhs = xT[:, 512 * (q - 12):512 * (q - 11)]
            nc.tensor.matmul(out=pstat[0:q + 1, :], lhsT=sel[:, 127 - q:128],
                             rhs=rhs, start=(q == 14), stop=(q == 0),
                             skip_group_check=True)
        st = stp.tile([15, 512], DT.float32, tag="st", name=f"st{s}")
        nc.scalar.copy(out=st, in_=pstat[0:15, :])
        return st, pstat

    def emit_gather(s, st, pstat):
        """stat transposes (PE) + strided scatter into q-major stats (ACT)."""
        for t in range(4):
            nc.tensor.transpose(
                out=pstat[:, 15 * t:15 * (t + 1)],
                in_=st[0:15, 128 * t:128 * (t + 1)], identity=idf[0:15, 0:15])
        dst = _bv(stats[:], [[W, 15], [1, 4]], extra_offset=4 * s)
        srcv = _bv(pstat[:], [[1, 15], [15, 4]])
        nc.vector.tensor_copy(out=dst, in_=srcv)

    PF = 3
    CS = 2  # cast lookahead
    PUMP = 8
    _DONE = object()
    gens = []
    live = {}
    for s in range(min(PF, n_supers)):
        if s < CS:
            # ramp: direct SWDGE cast-load for y (no SP round trip)
            xb = loads.tile([128, 4 * F], DT.bfloat16, tag="xb", name=f"xb{s}")
            yb = casts.tile([128, 4 * F], DT.bfloat16, tag="yb", name=f"yb{s}")
            nc.gpsimd.dma_start(
                out=xb[:].rearrange("p (t f) -> p t f", t=4),
                in_=x_d[512 * s:512 * (s + 1), :].rearrange("(t p) f -> p t f", p=128))
            nc.gpsimd.dma_start(
                out=yb[:].rearrange("p (t f) -> p t f", t=4),
                in_=y_d[512 * s:512 * (s + 1), :].rearrange("(t p) f -> p t f", p=128))
            live[s] = {"ld": (xb, None), "yb": yb}
        else:
            live[s] = {"ld": emit_loads(s, y_eng=nc.scalar if s == CS else nc.sync)}
    for s in range(n_supers + 4):
        if s < n_supers:
            cur = live[s]
            cur["pT"] = emit_trans(s, cur["ld"][0], cur["yb"])
        if s - 1 >= 0 and s - 1 < n_supers:
            st1 = live[s - 1]
            st1["pp"] = emit_pp(s - 1, st1["pT"])
        if s + CS < n_supers:
            live[s + CS]["yb"] = emit_cast(s + CS, live[s + CS]["ld"][1])
        if s + PF < n_supers:
            live[s + PF] = {"ld": emit_loads(s + PF)}
        if s - 2 >= 0 and s - 2 < n_supers:
            st2 = live[s - 2]
            st2["st"] = emit_stats(s - 2, *st2["pp"])
        if s - 3 >= 0 and s - 3 < n_supers:
            st3 = live[s - 3]
            emit_gather(s - 3, st3["st"][0], st3["st"][1])
            del live[s - 3]
        if s - 3 == n_supers // 2 - 1 and n_supers >= 8:
            gens.append(_final_math_gen(nc, fin, stats, loss, W, 0, W // 2, "A"))
        if s - 3 == 3 * n_supers // 4 - 1 and n_supers >= 8:
            gens.append(_final_math_gen(nc, fin, stats, loss, W, W // 2, 3 * W // 4, "Q3"))
        for g in list(gens):
            for _ in range(PUMP):
                if next(g, _DONE) is _DONE:
                    gens.remove(g)
                    break

    nc.sync.dma_start(out=ssq_d, in_=ssq_cols)
    for g in gens:
        for _ in g:
            pass
    c_lo = 3 * W // 4 if n_supers >= 8 else 0
    if c_lo > 0:
        nc.sync.dma_start(out=loss_d[:, 0:c_lo], in_=loss[:, 0:c_lo])
    for _ in _final_math_gen(nc, fin, stats, loss, W, c_lo, W, "B", pool_tt=True):
        pass
    nc.sync.dma_start(out=loss_d[:, c_lo:W], in_=loss[:, c_lo:W])


def _final_math_gen(nc, fin, stats, loss, W, c0, c1, sfx, pool_tt=False):
    """Final per-cloud math over loss columns [c0, c1), emitted lazily.

    Yields after each instruction so the driver can interleave emission with
    the main loop (keeps the DVE queue from head-blocking on a burst).
    ``pool_tt`` routes the det/M chains to the Pool engine (for the tail,
    when Pool is idle).
    """
    f32 = DT.float32
    V = nc.vector
    S = nc.scalar
    P = nc.gpsimd if pool_tt else nc.vector
    Wd = c1 - c0

    def T_(tag, mult=1):
        return fin.tile([128, mult * Wd], f32, tag=tag + sfx, name=tag + sfx)

    stats_ap = stats[:]

    def q_ap(q, n=1):
        if Wd == W:
            return stats[:, q * W:(q + n) * W]
        return _bv(stats_ap, [[W, n], [1, Wd]], extra_offset=q * W + c0)

    def q_view(q, dims):
        return _bv(stats_ap, dims, extra_offset=q * W + c0)

    inv_n = -1.0 / 128.0

    # --- C = G - sx sy^T / N ---
    sp9 = T_("sp9", 9)
    sx_b = q_view(12, [[W, 3], [0, 3], [1, Wd]])   # (j, k, T)
    sy_b = q_view(9, [[0, 3], [W, 3], [1, Wd]])
    V.tensor_tensor(out=sp9[:].rearrange("p (j k t) -> p j k t", j=3, k=3),
                    in0=sx_b, in1=sy_b, op=ALU.mult)
    yield
    C = T_("C", 9)
    V.scalar_tensor_tensor(out=C, in0=sp9, scalar=inv_n, in1=q_ap(0, 9),
                           op0=ALU.mult, op1=ALU.add)
    yield
    Cap = C[:]

    def C_(j, k):
        return C[:, (3 * j + k) * Wd:(3 * j + k + 1) * Wd]

    # --- l2 deficit: -(|sx|^2 + |sy|^2)/N ---
    sq6 = T_("sq6", 6)
    V.tensor_tensor(out=sq6, in0=q_ap(9, 6), in1=q_ap(9, 6), op=ALU.mult)
    yield
    l2p = T_("l2p")
    V.tensor_reduce(out=l2p, in_=_bv(sq6[:], [[1, Wd], [Wd, 6]]), axis=mybir.AxisListType.X,
                    op=ALU.add)
    yield

    # --- I1 = sum C^2 ---
    csq = T_("csq", 9)
    V.tensor_tensor(out=csq, in0=C, in1=C, op=ALU.mult)
    yield
    I1 = T_("I1")
    V.tensor_reduce(out=I1, in_=_bv(csq[:], [[1, Wd], [Wd, 9]]), axis=mybir.AxisListType.X,
                    op=ALU.add)
    yield

    # --- M = C^T C via 3 outer products; trM2 = sum M^2 ---
    M9 = T_("M9", 9)
    t9 = T_("t9", 9)
    for j in range(3):
        ca = _bv(Cap, [[Wd, 3], [0, 3], [1, Wd]], extra_offset=3 * j * Wd)
        cb = _bv(Cap, [[0, 3], [Wd, 3], [1, Wd]], extra_offset=3 * j * Wd)
        dstv = (M9 if j == 0 else t9)[:].rearrange("p (a b t) -> p a b t", a=3, b=3)
        P.tensor_tensor(out=dstv, in0=ca, in1=cb, op=ALU.mult)
        yield
        if j > 0:
            P.tensor_tensor(out=M9, in0=M9, in1=t9, op=ALU.add)
            yield
    msq = T_("msq", 9)
    P.tensor_tensor(out=msq, in0=M9, in1=M9, op=ALU.mult)
    yield
    trM2 = T_("trM2")
    V.tensor_reduce(out=trM2, in_=_bv(msq[:], [[1, Wd], [Wd, 9]]), axis=mybir.AxisListType.X,
                    op=ALU.add)
    yield

    # --- tdif = I1^2 - trM2  (I2 = tdif/2 folded into usq2) ---
    I1sq = T_("I1sq")
    V.tensor_tensor(out=I1sq, in0=I1, in1=I1, op=ALU.mult)
    yield
    tdif = T_("tdif")
    V.tensor_tensor(out=tdif, in0=I1sq, in1=trM2, op=ALU.subtract)
    yield

    # --- det(C) ---
    ta = T_("ta")
    tb = T_("tb")
    det = T_("det")
    cof = T_("cof")
    P.tensor_tensor(out=ta, in0=C_(1, 1), in1=C_(2, 2), op=ALU.mult)
    yield
    P.tensor_tensor(out=tb, in0=C_(1, 2), in1=C_(2, 1), op=ALU.mult)
    yield
    P.tensor_tensor(out=cof, in0=ta, in1=tb, op=ALU.subtract)
    yield
    P.tensor_tensor(out=det, in0=C_(0, 0), in1=cof, op=ALU.mult)
    yield
    P.tensor_tensor(out=ta, in0=C_(1, 0), in1=C_(2, 2), op=ALU.mult)
    yield
    P.tensor_tensor(out=tb, in0=C_(1, 2), in1=C_(2, 0), op=ALU.mult)
    yield
    P.tensor_tensor(out=cof, in0=ta, in1=tb, op=ALU.subtract)
    yield
    P.tensor_tensor(out=cof, in0=C_(0, 1), in1=cof, op=ALU.mult)
    yield
    P.tensor_tensor(out=det, in0=det, in1=cof, op=ALU.subtract)
    yield
    P.tensor_tensor(out=ta, in0=C_(1, 0), in1=C_(2, 1), op=ALU.mult)
    yield
    P.tensor_tensor(out=tb, in0=C_(1, 1), in1=C_(2, 0), op=ALU.mult)
    yield
    P.tensor_tensor(out=cof, in0=ta, in1=tb, op=ALU.subtract)
    yield
    P.tensor_tensor(out=cof, in0=C_(0, 2), in1=cof, op=ALU.mult)
    yield
    P.tensor_tensor(out=det, in0=det, in1=cof, op=ALU.add)
    yield
    e3 = T_("e3")
    S.activation(out=e3, in_=det, func=ACT.Abs)
    yield

    # --- normalize: u = 3/I1 ---
    I1c = T_("I1c")
    V.tensor_scalar_max(I1c, I1, 1e-20)
    yield
    u = T_("u")
    V.reciprocal(out=u, in_=I1c)
    yield
    V.tensor_scalar_mul(u, u, 3.0)
    yield
    usq2 = T_("usq2")
    V.tensor_tensor(out=usq2, in0=u, in1=u, op=ALU.mult)
    yield
    V.tensor_scalar_mul(usq2, usq2, 0.5)
    yield
    I2n = T_("I2n")
    V.tensor_tensor(out=I2n, in0=tdif, in1=usq2, op=ALU.mult)
    yield
    V.tensor_scalar_max(I2n, I2n, 0.0)
    yield
    su = T_("su")
    S.activation(out=su, in_=u, func=ACT.Sqrt)
    yield
    e3n = T_("e3n")
    V.tensor_tensor(out=e3n, in0=e3, in1=u, op=ALU.mult)
    yield
    V.tensor_tensor(out=e3n, in0=e3n, in1=su, op=ALU.mult)
    yield
    E8 = T_("E8")
    V.tensor_scalar_mul(E8, e3n, 8.0)
    yield
    c0t = T_("c0t")
    V.tensor_scalar(out=c0t, in0=I2n, scalar1=-4.0, scalar2=9.0, op0=ALU.mult, op1=ALU.add)
    yield

    # --- Newton init: n = sqrt(3 + 2*sqrt(I2n)) ---
    b3 = fin.tile([128, 1], f32, tag="b3" + sfx, name="b3" + sfx)
    V.memset(b3, 3.0)
    yield
    sqi = T_("sqi")
    S.activation(out=sqi, in_=I2n, func=ACT.Sqrt)
    yield
    n = T_("n")
    S.activation(out=n, in_=sqi, func=ACT.Sqrt, bias=b3[:, 0:1], scale=2.0)
    yield

    # --- Newton iterations on n^4 - 6n^2 - 8 e3n n + c0 ---
    t1 = T_("t1")
    t3 = T_("t3")
    s1 = T_("s1")
    f0 = T_("f0")
    fv = T_("fv")
    av = T_("av")
    fp = T_("fp")
    rp = T_("rp")
    dd = T_("dd")
    for it in range(2):
        V.tensor_tensor(out=t1, in0=n, in1=n, op=ALU.mult)
        yield
        V.scalar_tensor_tensor(out=t3, in0=t1, scalar=-6.0, in1=n,
                               op0=ALU.add, op1=ALU.mult)  # (n^2-6)*n
        yield
        V.scalar_tensor_tensor(out=s1, in0=E8, scalar=-1.0, in1=t3,
                               op0=ALU.mult, op1=ALU.add)  # t3 - E8
        yield
        V.tensor_tensor(out=f0, in0=s1, in1=n, op=ALU.mult)
        yield
        V.tensor_tensor(out=fv, in0=f0, in1=c0t, op=ALU.add)
        yield
        V.scalar_tensor_tensor(out=av, in0=n, scalar=3.0, in1=t3,
                               op0=ALU.mult, op1=ALU.add)  # n^3 - 3n
        yield
        V.scalar_tensor_tensor(out=fp, in0=av, scalar=4.0, in1=E8,
                               op0=ALU.mult, op1=ALU.subtract)  # 4n^3-12n-8e
        yield
        V.tensor_scalar_max(fp, fp, 1e-5)
        yield
        V.reciprocal(out=rp, in_=fp)
        yield
        V.tensor_tensor(out=dd, in0=fv, in1=rp, op=ALU.mult)
        yield
        V.tensor_tensor(out=n, in0=n, in1=dd, op=ALU.subtract)
        yield
        if it == 0:
            V.tensor_scalar_min(n, n, 3.01)
            yield
            V.tensor_scalar_max(n, n, 1.70)
            yield

    # --- un-normalize: s = sqrt(I1/3) with one Newton sqrt refinement ---
    vv = T_("vv")
    V.tensor_scalar_mul(vv, I1, 1.0 / 3.0)
    yield
    V.tensor_scalar_max(vv, vv, 1e-30)
    yield
    s0 = T_("s0")
    S.activation(out=s0, in_=vv, func=ACT.Sqrt)
    yield

    # --- loss_c = -(|sx|^2+|sy|^2)/N - 2 * n * s ---
    V.tensor_tensor(out=n, in0=n, in1=s0, op=ALU.mult)
    yield
    l2s = T_("l2s")
    V.tensor_scalar_mul(l2s, l2p, inv_n)
    yield
    V.scalar_tensor_tensor(out=loss[:, c0:c1], in0=n, scalar=-2.0, in1=l2s,
                           op0=ALU.mult, op1=ALU.add)
    yield


# ---------------------------------------------------------------------------# ---------------------------------------------------------------------------
# host glue
# ---------------------------------------------------------------------------


class Runner:
    """Cached jitted shard_map executor for repeated invocations (timing)."""

    def __init__(self, nc, n_cores=N_CORES):
        import jax
        from jax.experimental.shard_map import shard_map
        from jax.sharding import Mesh, PartitionSpec
        from concourse import bass2jax
        from concourse import mybir as _mybir

        bass2jax.install_neuronx_cc_hook()
        self.nc = nc
        self.n_cores = n_cores
        partition_name = nc.partition_id_tensor.name if nc.partition_id_tensor else None
        in_names, out_names, out_avals, zero_outs = [], [], [], []
        for alloc in nc.m.functions[0].allocations:
            if not isinstance(alloc, _mybir.MemoryLocationSet):
                continue
            name = alloc.memorylocations[0].name
            if alloc.kind == "ExternalInput":
                if name != partition_name:
                    in_names.append(name)
            elif alloc.kind == "ExternalOutput":
                out_names.append(name)
                shape = tuple(alloc.tensor_shape)
                dtype = _mybir.dt.np(alloc.dtype)
                out_avals.append(jax.core.ShapedArray(shape, dtype))
                zero_outs.append(np.zeros(shape, dtype))
        self.in_names = list(in_names)
        self.out_names = out_names
        self.zero_outs = zero_outs
        n_params = len(in_names)
        n_outs = len(out_avals)
        all_in_names = in_names + out_names
        if partition_name is not None:
            all_in_names = all_in_names + [partition_name]

        def _body(*args):
            operands = list(args)
            if partition_name is not None:
                operands.append(bass2jax.partition_id_tensor())
            outs = bass2jax._bass_exec_p.bind(
                *operands,
                out_avals=tuple(out_avals),
                in_names=tuple(all_in_names),
                out_names=tuple(out_names),
                lowering_input_output_aliases=(),
                sim_require_finite=True,
                sim_require_nnan=True,
                nc=nc,
            )
            return tuple(outs)

        devices = jax.devices()[:n_cores]
        mesh = Mesh(np.asarray(devices), ("core",))
        self.mesh = mesh
        in_specs = (PartitionSpec("core"),) * (n_params + n_outs)
        out_specs = (PartitionSpec("core"),) * n_outs
        self.fn = jax.jit(
            shard_map(_body, mesh=mesh, in_specs=in_specs, out_specs=out_specs,
                      check_rep=False),
            keep_unused=True,
        )

    def prep(self, in_maps, device_put=True):
        """in_maps: list of per-core dicts -> concatenated arg list (device-resident)."""
        concat = [
            np.concatenate([np.asarray(in_maps[c][n]) for c in range(self.n_cores)], axis=0)
            for n in self.in_names
        ]
        concat += [
            np.zeros((self.n_cores * z.shape[0], *z.shape[1:]), z.dtype)
            for z in self.zero_outs
        ]
        if device_put:
            import jax
            from jax.sharding import NamedSharding, PartitionSpec

            sh = NamedSharding(self.mesh, PartitionSpec("core"))
            concat = [jax.device_put(a, sh) for a in concat]
            jax.block_until_ready(concat)
        return concat

    def __call__(self, args):
        return self.fn(*args)


_NC_CACHE = {}


def _get_nc(b_core=B_CORE):
    if b_core not in _NC_CACHE:
        _NC_CACHE[b_core] = build_kernel(b_core)
    return _NC_CACHE[b_core]


def _consts():
    sel = np.zeros((128, 128), ml_dtypes.bfloat16)
    sel[:, 127] = 1.0
    idb = np.eye(128, dtype=ml_dtypes.bfloat16)
    idf = np.eye(128, dtype=np.float32)
    return sel, idb, idf


def run_cores(x, y, b_core=B_CORE, n_cores=N_CORES, nc=None):
    """x, y: (n_cores*b_core, 128, 3) float32 -> list of per-core (loss, ssq) grids."""
    if nc is None:
        nc = _get_nc(b_core)
    sel, idb, idf = _consts()
    xs = np.ascontiguousarray(x, dtype=np.float32).reshape(n_cores, b_core, F)
    ys = np.ascontiguousarray(y, dtype=np.float32).reshape(n_cores, b_core, F)
    in_maps = [
        {"x": xs[c], "y": ys[c], "sel": sel, "idb": idb, "idf": idf}
        for c in range(n_cores)
    ]
    res = run_bass_kernel_spmd(nc, in_maps, core_ids=list(range(n_cores)))
    return [(res.results[c]["loss"], res.results[c]["ssq"]) for c in range(n_cores)]


def kernel(x, y):
    """Full-input entry point: x, y (65536, 128, 3) float32 -> scalar float32."""
    grids = run_cores(np.asarray(x), np.asarray(y))
    total = sum(
        g.astype(np.float64).sum() + q.astype(np.float64).sum() for g, q in grids
    )
    return np.float32(total / (B_TOTAL * N_PTS * 3))


# revision 34
# speedup vs baseline: 1.1351x; 1.1351x over previous
"""Kabsch loss kernel for Trainium2 (8 NeuronCores, data-parallel over batch).

Math: for each batch b (128 points, 3 dims):
  loss_b = ||xc||_F^2 + ||yc||_F^2 - 2 * nuclear_norm(C),  C = xc^T yc (3x3)
because R = U Vh from SVD(C) gives tr(R^T C) = sum of singular values.
nuclear_norm(C) is computed from the invariants of C (I1=||C||_F^2,
I2 = 2nd invariant of C^T C, e3=|det C|) via Newton iteration on the quartic
  n^4 - 2*I1*n^2 - 8*e3*n + (I1^2 - 4*I2) = 0   (largest root = sigma1+sigma2+sigma3)
normalized so I1 -> 3.

Dataflow per super-tile (512 clouds), software-pipelined so every consumer
runs one iteration behind its producer (in-order engine queues never stall on
same-iteration data):
  x: SWDGE cast-load HBM f32 -> SBUF bf16;  y: HWDGE f32 (SP) + SWDGE SBUF cast
  PE: 24 transposes -> point-major planes in one [128,3072] PSUM tile (x|y)
  ACT/DVE: evacuate planes to SBUF xT/yT (each copy has a single PSUM input --
    hardware allows at most one PSUM operand, and GPSIMD cannot touch PSUM)
  DVE/Pool: 3 triple-wide products x_j (x) [y0|y1|y2] -> [128,9,512] bf16
  PE: 15-matmul ones-chain (sel trick) -> per-cloud G/sy/sx rows in PSUM
  ACT: ssq-x accumulator; DVE: ssq-y accumulator
  PE: 4 [15,128]->[128,15] stat transposes;  ACT: strided copy to q-major grid
Final math (Newton quartic, 2 iters) runs as two [128,32] column halves: the
first is pumped a few ops per iteration into engine idle slots mid-loop, the
second runs at the tail with its det/M chains routed to the then-idle Pool.
"""

import sys

sys.path.insert(0, "/opt/trn_rl_repo")

from contextlib import ExitStack

import numpy as np
import ml_dtypes

import concourse.bass as bass
import concourse.tile as tile
from concourse import bacc, mybir
from concourse.bass_utils import run_bass_kernel_spmd

DT = mybir.dt
ALU = mybir.AluOpType
ACT = mybir.ActivationFunctionType

N_CORES = 8
B_TOTAL = 65536
N_PTS = 128
B_CORE = B_TOTAL // N_CORES  # 8192
F = N_PTS * 3  # 384


def _bv(base_ap, dims, extra_offset=0):
    """Build an AP reusing base_ap's partition dim + offset with custom free dims."""
    return bass.AP(base_ap.tensor, base_ap.offset + extra_offset,
                   [list(base_ap.ap[0])] + [list(d) for d in dims])


def build_kernel(b_core=B_CORE, n_cores=N_CORES):
    n_tiles = b_core // 128
    assert n_tiles % 4 == 0
    n_supers = n_tiles // 4
    W = n_tiles  # loss columns (single final pass)

    nc = bacc.Bacc("TRN2", target_bir_lowering=False, debug=False, num_devices=n_cores)
    x_d = nc.dram_tensor("x", [b_core, F], DT.float32, kind="ExternalInput").ap()
    y_d = nc.dram_tensor("y", [b_core, F], DT.float32, kind="ExternalInput").ap()
    sel_d = nc.dram_tensor("sel", [128, 128], DT.bfloat16, kind="ExternalInput").ap()
    idb_d = nc.dram_tensor("idb", [128, 128], DT.bfloat16, kind="ExternalInput").ap()
    idf_d = nc.dram_tensor("idf", [128, 128], DT.float32, kind="ExternalInput").ap()
    loss_d = nc.dram_tensor("loss", [128, W], DT.float32, kind="ExternalOutput").ap()
    ssq_d = nc.dram_tensor("ssq", [128, 2 * n_supers], DT.float32, kind="ExternalOutput").ap()

    with tile.TileContext(nc) as tc:
        with ExitStack() as ctx:
            _kabsch(ctx, tc, x_d, y_d, sel_d, idb_d, idf_d, loss_d, ssq_d,
                    n_tiles, n_supers, W)
    nc.compile()
    return nc


def _kabsch(ctx, tc, x_d, y_d, sel_d, idb_d, idf_d, loss_d, ssq_d,
            n_tiles, n_supers, W):
    nc = tc.nc
    singles = ctx.enter_context(tc.tile_pool(name="singles", bufs=1))
    loads = ctx.enter_context(tc.tile_pool(name="loads", bufs=5))
    casts = ctx.enter_context(tc.tile_pool(name="casts", bufs=5))
    prods = ctx.enter_context(tc.tile_pool(name="prods", bufs=4))
    stp = ctx.enter_context(tc.tile_pool(name="stp", bufs=4))
    xtp = ctx.enter_context(tc.tile_pool(name="xtp", bufs=3))
    ytp = ctx.enter_context(tc.tile_pool(name="ytp", bufs=3))
    junkp = ctx.enter_context(tc.tile_pool(name="junkp", bufs=3))
    fin = ctx.enter_context(tc.tile_pool(name="fin", bufs=1))
    psum = ctx.enter_context(tc.tile_pool(name="psum", bufs=2, space="PSUM"))

    # constants
    sel = singles.tile([128, 128], DT.bfloat16, tag="sel")
    idb = singles.tile([128, 128], DT.bfloat16, tag="idb")
    idf = singles.tile([128, 128], DT.float32, tag="idf")
    nc.sync.dma_start(out=sel, in_=sel_d)
    nc.sync.dma_start(out=idb, in_=idb_d)
    nc.sync.dma_start(out=idf, in_=idf_d)
    ones = sel[:, 127:128]  # [128,1] bf16 ones column

    # persistent accumulators
    ssq_cols = singles.tile([128, 2 * n_supers], DT.float32, tag="ssq_cols", name="ssq_cols")
    stats = singles.tile([128, 15 * W], DT.float32, tag="stats", name="stats")
    loss = singles.tile([128, W], DT.float32, tag="loss", name="loss")

    def emit_loads(s, y_eng=None):
        xb = loads.tile([128, 4 * F], DT.bfloat16, tag="xb", name=f"xb{s}")
        yf = loads.tile([128, 4 * F], DT.float32, tag="yf", name=f"yf{s}")
        nc.gpsimd.dma_start(
            out=xb[:].rearrange("p (t f) -> p t f", t=4),
            in_=x_d[512 * s:512 * (s + 1), :].rearrange("(t p) f -> p t f", p=128))
        (y_eng or nc.sync).dma_start(
            out=yf[:].rearrange("p (t f) -> p t f", t=4),
            in_=y_d[512 * s:512 * (s + 1), :].rearrange("(t p) f -> p t f", p=128))
        return xb, yf

    def emit_cast(s, yf):
        yb = casts.tile([128, 4 * F], DT.bfloat16, tag="yb", name=f"yb{s}")
        nc.gpsimd.dma_start(out=yb, in_=yf)  # SBUF->SBUF cast
        return yb

    def emit_trans(s, xb, yb):
        """transposes (PE) + ssq (ACT) for super s. pT layout: x0|x1|x2|y0|y1|y2."""
        jx = junkp.tile([128, 4 * F], DT.bfloat16, tag="jx")
        jy = junkp.tile([128, 4 * F], DT.bfloat16, tag="jy")
        pT = psum.tile([128, 3072], DT.bfloat16, tag="pT", name=f"pT{s}")
        for t in range(4):
            for j in range(3):
                nc.tensor.transpose(
                    out=pT[:, 512 * j + 128 * t:512 * j + 128 * (t + 1)],
                    in_=_bv(xb[:], [[3, 128]], extra_offset=F * t + j), identity=idb)
                nc.tensor.transpose(
                    out=pT[:, 1536 + 512 * j + 128 * t:1536 + 512 * j + 128 * (t + 1)],
                    in_=_bv(yb[:], [[3, 128]], extra_offset=F * t + j), identity=idb)
        nc.scalar.activation(out=jx, in_=xb, func=ACT.Square,
                             accum_out=ssq_cols[:, s:s + 1])
        nc.scalar.activation(out=jy, in_=yb, func=ACT.Square,
                             accum_out=ssq_cols[:, n_supers + s:n_supers + s + 1])
        return pT

    def emit_pp(s, pT):
        """plane evacuation (ACT/DVE, one-PSUM-input copies) + products (DVE)."""
        xT = xtp.tile([128, 1536], DT.bfloat16, tag="xT", name=f"xT{s}")
        yT = ytp.tile([128, 1536], DT.bfloat16, tag="yT", name=f"yT{s}")
        nc.vector.tensor_copy(out=xT, in_=pT[:, 0:1536])
        nc.vector.tensor_copy(out=yT, in_=pT[:, 1536:3072])
        prod = prods.tile([128, 9, 512], DT.bfloat16, tag="prod", name=f"prod{s}")
        xTap = xT[:]
        for j in range(2, -1, -1):
            xj_rep = _bv(xTap, [[0, 3], [1, 512]], extra_offset=512 * j)
            on_pool = j == 2 or (j == 1 and s % 2 == 1)
            eng = nc.gpsimd if on_pool else nc.vector
            eng.tensor_mul(
                prod[:, 3 * j:3 * j + 3, :].rearrange("p a c -> p (a c)"),
                xj_rep, yT)
        return prod, xT, yT

    def emit_stats(s, prod, xT, yT):
        """15-matmul chain into pstat rows 0..14 (PE), ACT evac to st."""
        pstat = psum.tile([128, 512], DT.float32, tag="pstat", name=f"pstat{s}")
        for q in range(14, -1, -1):
            if q < 9:
                rhs = prod[:, q, :]
            elif q < 12:
                rhs = yT[:, 512 * (q - 9):512 * (q - 8)]
            else:
                rhs = xT[:, 512 * (q - 12):512 * (q - 11)]
            nc.tensor.matmul(out=pstat[0:q + 1, :], lhsT=sel[:, 127 - q:128],
                             rhs=rhs, start=(q == 14), stop=(q == 0),
                             skip_group_check=True)
        st = stp.tile([15, 512], DT.float32, tag="st", name=f"st{s}")
        nc.scalar.copy(out=st, in_=pstat[0:15, :])
        return st, pstat

    def emit_gather(s, st, pstat):
        """stat transposes (PE) + strided scatter into q-major stats (ACT)."""
        for t in range(4):
            nc.tensor.transpose(
                out=pstat[:, 15 * t:15 * (t + 1)],
                in_=st[0:15, 128 * t:128 * (t + 1)], identity=idf[0:15, 0:15])
        dst = _bv(stats[:], [[W, 15], [1, 4]], extra_offset=4 * s)
        srcv = _bv(pstat[:], [[1, 15], [15, 4]])
        nc.vector.tensor_copy(out=dst, in_=srcv)

    PF = 3
    CS = 2  # cast lookahead
    PUMP = 8
    _DONE = object()
    gens = []
    live = {}
    for s in range(min(PF, n_supers)):
        if s < CS:
            # ramp: direct SWDGE cast-load for y (no SP round trip)
            xb = loads.tile([128, 4 * F], DT.bfloat16, tag="xb", name=f"xb{s}")
            yb = casts.tile([128, 4 * F], DT.bfloat16, tag="yb", name=f"yb{s}")
            nc.gpsimd.dma_start(
                out=xb[:].rearrange("p (t f) -> p t f", t=4),
                in_=x_d[512 * s:512 * (s + 1), :].rearrange("(t p) f -> p t f", p=128))
            nc.gpsimd.dma_start(
                out=yb[:].rearrange("p (t f) -> p t f", t=4),
                in_=y_d[512 * s:512 * (s + 1), :].rearrange("(t p) f -> p t f", p=128))
            live[s] = {"ld": (xb, None), "yb": yb}
        else:
            live[s] = {"ld": emit_loads(s, y_eng=nc.scalar if s == CS else nc.sync)}
    for s in range(n_supers + 4):
        if s < n_supers:
            cur = live[s]
            cur["pT"] = emit_trans(s, cur["ld"][0], cur["yb"])
        if s - 1 >= 0 and s - 1 < n_supers:
            st1 = live[s - 1]
            st1["pp"] = emit_pp(s - 1, st1["pT"])
        if s + CS < n_supers:
            live[s + CS]["yb"] = emit_cast(s + CS, live[s + CS]["ld"][1])
        if s + PF < n_supers:
            live[s + PF] = {"ld": emit_loads(s + PF)}
        if s - 2 >= 0 and s - 2 < n_supers:
            st2 = live[s - 2]
            st2["st"] = emit_stats(s - 2, *st2["pp"])
        if s - 3 >= 0 and s - 3 < n_supers:
            st3 = live[s - 3]
            emit_gather(s - 3, st3["st"][0], st3["st"][1])
            del live[s - 3]
        if s - 3 == n_supers // 2 - 1 and n_supers >= 8:
            gens.append(_final_math_gen(nc, fin, stats, loss, W, 0, W // 2, "A"))
        if s - 3 == 3 * n_supers // 4 - 1 and n_supers >= 8:
            gens.append(_final_math_gen(nc, fin, stats, loss, W, W // 2, 3 * W // 4, "Q3"))
        for g in list(gens):
            for _ in range(PUMP):
                if next(g, _DONE) is _DONE:
                    gens.remove(g)
                    break

    nc.sync.dma_start(out=ssq_d, in_=ssq_cols)
    for g in gens:
        for _ in g:
            pass
    c_lo = 3 * W // 4 if n_supers >= 8 else 0
    if c_lo > 0:
        nc.sync.dma_start(out=loss_d[:, 0:c_lo], in_=loss[:, 0:c_lo])
    for _ in _final_math_gen(nc, fin, stats, loss, W, c_lo, W, "B", pool_tt=True):
        pass
    nc.sync.dma_start(out=loss_d[:, c_lo:W], in_=loss[:, c_lo:W])


def _final_math_gen(nc, fin, stats, loss, W, c0, c1, sfx, pool_tt=False):
    """Final per-cloud math over loss columns [c0, c1), emitted lazily.

    Yields after each instruction so the driver can interleave emission with
    the main loop (keeps the DVE queue from head-blocking on a burst).
    ``pool_tt`` routes the det/M chains to the Pool engine (for the tail,
    when Pool is idle).
    """
    f32 = DT.float32
    V = nc.vector
    S = nc.scalar
    P = nc.gpsimd if pool_tt else nc.vector
    Wd = c1 - c0

    def T_(tag, mult=1):
        return fin.tile([128, mult * Wd], f32, tag=tag + sfx, name=tag + sfx)

    stats_ap = stats[:]

    def q_ap(q, n=1):
        if Wd == W:
            return stats[:, q * W:(q + n) * W]
        return _bv(stats_ap, [[W, n], [1, Wd]], extra_offset=q * W + c0)

    def q_view(q, dims):
        return _bv(stats_ap, dims, extra_offset=q * W + c0)

    inv_n = -1.0 / 128.0

    # --- C = G - sx sy^T / N ---
    sp9 = T_("sp9", 9)
    sx_b = q_view(12, [[W, 3], [0, 3], [1, Wd]])   # (j, k, T)
    sy_b = q_view(9, [[0, 3], [W, 3], [1, Wd]])
    V.tensor_tensor(out=sp9[:].rearrange("p (j k t) -> p j k t", j=3, k=3),
                    in0=sx_b, in1=sy_b, op=ALU.mult)
    yield
    C = T_("C", 9)
    V.scalar_tensor_tensor(out=C, in0=sp9, scalar=inv_n, in1=q_ap(0, 9),
                           op0=ALU.mult, op1=ALU.add)
    yield
    Cap = C[:]

    def C_(j, k):
        return C[:, (3 * j + k) * Wd:(3 * j + k + 1) * Wd]

    # --- l2 deficit: -(|sx|^2 + |sy|^2)/N ---
    sq6 = T_("sq6", 6)
    V.tensor_tensor(out=sq6, in0=q_ap(9, 6), in1=q_ap(9, 6), op=ALU.mult)
    yield
    l2p = T_("l2p")
    V.tensor_reduce(out=l2p, in_=_bv(sq6[:], [[1, Wd], [Wd, 6]]), axis=mybir.AxisListType.X,
                    op=ALU.add)
    yield

    # --- I1 = sum C^2 ---
    csq = T_("csq", 9)
    V.tensor_tensor(out=csq, in0=C, in1=C, op=ALU.mult)
    yield
    I1 = T_("I1")
    V.tensor_reduce(out=I1, in_=_bv(csq[:], [[1, Wd], [Wd, 9]]), axis=mybir.AxisListType.X,
                    op=ALU.add)
    yield

    # --- M = C^T C via 3 outer products; trM2 = sum M^2 ---
    M9 = T_("M9", 9)
    t9 = T_("t9", 9)
    for j in range(3):
        ca = _bv(Cap, [[Wd, 3], [0, 3], [1, Wd]], extra_offset=3 * j * Wd)
        cb = _bv(Cap, [[0, 3], [Wd, 3], [1, Wd]], extra_offset=3 * j * Wd)
        dstv = (M9 if j == 0 else t9)[:].rearrange("p (a b t) -> p a b t", a=3, b=3)
        P.tensor_tensor(out=dstv, in0=ca, in1=cb, op=ALU.mult)
        yield
        if j > 0:
            P.tensor_tensor(out=M9, in0=M9, in1=t9, op=ALU.add)
            yield
    msq = T_("msq", 9)
    P.tensor_tensor(out=msq, in0=M9, in1=M9, op=ALU.mult)
    yield
    trM2 = T_("trM2")
    V.tensor_reduce(out=trM2, in_=_bv(msq[:], [[1, Wd], [Wd, 9]]), axis=mybir.AxisListType.X,
                    op=ALU.add)
    yield

    # --- tdif = I1^2 - trM2  (I2 = tdif/2 folded into usq2) ---
    I1sq = T_("I1sq")
    V.tensor_tensor(out=I1sq, in0=I1, in1=I1, op=ALU.mult)
    yield
    tdif = T_("tdif")
    V.tensor_tensor(out=tdif, in0=I1sq, in1=trM2, op=ALU.subtract)
    yield

    # --- det(C) ---
    ta = T_("ta")
    tb = T_("tb")
    det = T_("det")
    cof = T_("cof")
    P.tensor_tensor(out=ta, in0=C_(1, 1), in1=C_(2, 2), op=ALU.mult)
    yield
    P.tensor_tensor(out=tb, in0=C_(1, 2), in1=C_(2, 1), op=ALU.mult)
    yield
    P.tensor_tensor(out=cof, in0=ta, in1=tb, op=ALU.subtract)
    yield
    P.tensor_tensor(out=det, in0=C_(0, 0), in1=cof, op=ALU.mult)
    yield
    P.tensor_tensor(out=ta, in0=C_(1, 0), in1=C_(2, 2), op=ALU.mult)
    yield
    P.tensor_tensor(out=tb, in0=C_(1, 2), in1=C_(2, 0), op=ALU.mult)
    yield
    P.tensor_tensor(out=cof, in0=ta, in1=tb, op=ALU.subtract)
    yield
    P.tensor_tensor(out=cof, in0=C_(0, 1), in1=cof, op=ALU.mult)
    yield
    P.tensor_tensor(out=det, in0=det, in1=cof, op=ALU.subtract)
    yield
    P.tensor_tensor(out=ta, in0=C_(1, 0), in1=C_(2, 1), op=ALU.mult)
    yield
    P.tensor_tensor(out=tb, in0=C_(1, 1), in1=C_(2, 0), op=ALU.mult)
    yield
    P.tensor_tensor(out=cof, in0=ta, in1=tb, op=ALU.subtract)
    yield
    P.tensor_tensor(out=cof, in0=C_(0, 2), in1=cof, op=ALU.mult)
    yield
    P.tensor_tensor(out=det, in0=det, in1=cof, op=ALU.add)
    yield
    e3 = T_("e3")
    S.activation(out=e3, in_=det, func=ACT.Abs)
    yield

    # --- normalize: u = 3/I1 ---
    I1c = T_("I1c")
    V.tensor_scalar_max(I1c, I1, 1e-20)
    yield
    u = T_("u")
    V.reciprocal(out=u, in_=I1c)
    yield
    V.tensor_scalar_mul(u, u, 3.0)
    yield
    usq2 = T_("usq2")
    V.tensor_tensor(out=usq2, in0=u, in1=u, op=ALU.mult)
    yield
    V.tensor_scalar_mul(usq2, usq2, 0.5)
    yield
    I2n = T_("I2n")
    V.tensor_tensor(out=I2n, in0=tdif, in1=usq2, op=ALU.mult)
    yield
    V.tensor_scalar_max(I2n, I2n, 0.0)
    yield
    su = T_("su")
    S.activation(out=su, in_=u, func=ACT.Sqrt)
    yield
    e3n = T_("e3n")
    V.tensor_tensor(out=e3n, in0=e3, in1=u, op=ALU.mult)
    yield
    V.tensor_tensor(out=e3n, in0=e3n, in1=su, op=ALU.mult)
    yield
    E8 = T_("E8")
    V.tensor_scalar_mul(E8, e3n, 8.0)
    yield
    c0t = T_("c0t")
    V.tensor_scalar(out=c0t, in0=I2n, scalar1=-4.0, scalar2=9.0, op0=ALU.mult, op1=ALU.add)
    yield

    # --- Newton init: n = sqrt(3 + 2*sqrt(I2n)) ---
    b3 = fin.tile([128, 1], f32, tag="b3" + sfx, name="b3" + sfx)
    V.memset(b3, 3.0)
    yield
    sqi = T_("sqi")
    S.activation(out=sqi, in_=I2n, func=ACT.Sqrt)
    yield
    n = T_("n")
    S.activation(out=n, in_=sqi, func=ACT.Sqrt, bias=b3[:, 0:1], scale=2.0)
    yield

    # --- Newton iterations on n^4 - 6n^2 - 8 e3n n + c0 ---
    t1 = T_("t1")
    t3 = T_("t3")
    s1 = T_("s1")
    f0 = T_("f0")
    fv = T_("fv")
    av = T_("av")
    fp = T_("fp")
    rp = T_("rp")
    dd = T_("dd")
    for it in range(2):
        V.tensor_tensor(out=t1, in0=n, in1=n, op=ALU.mult)
        yield
        V.scalar_tensor_tensor(out=t3, in0=t1, scalar=-6.0, in1=n,
                               op0=ALU.add, op1=ALU.mult)  # (n^2-6)*n
        yield
        V.scalar_tensor_tensor(out=s1, in0=E8, scalar=-1.0, in1=t3,
                               op0=ALU.mult, op1=ALU.add)  # t3 - E8
        yield
        V.tensor_tensor(out=f0, in0=s1, in1=n, op=ALU.mult)
        yield
        V.tensor_tensor(out=fv, in0=f0, in1=c0t, op=ALU.add)
        yield
        V.scalar_tensor_tensor(out=av, in0=n, scalar=3.0, in1=t3,
                               op0=ALU.mult, op1=ALU.add)  # n^3 - 3n
        yield
        V.scalar_tensor_tensor(out=fp, in0=av, scalar=4.0, in1=E8,
                               op0=ALU.mult, op1=ALU.subtract)  # 4n^3-12n-8e
        yield
        V.tensor_scalar_max(fp, fp, 1e-5)
        yield
        V.reciprocal(out=rp, in_=fp)
        yield
        V.tensor_tensor(out=dd, in0=fv, in1=rp, op=ALU.mult)
        yield
        V.tensor_tensor(out=n, in0=n, in1=dd, op=ALU.subtract)
        yield
        if it == 0:
            V.tensor_scalar_min(n, n, 3.01)
            yield
            V.tensor_scalar_max(n, n, 1.70)
            yield

    # --- un-normalize: s = sqrt(I1/3) with one Newton sqrt refinement ---
    vv = T_("vv")
    V.tensor_scalar_mul(vv, I1, 1.0 / 3.0)
    yield
    V.tensor_scalar_max(vv, vv, 1e-30)
    yield
    s0 = T_("s0")
    S.activation(out=s0, in_=vv, func=ACT.Sqrt)
    yield

    # --- loss_c = -(|sx|^2+|sy|^2)/N - 2 * n * s ---
    V.tensor_tensor(out=n, in0=n, in1=s0, op=ALU.mult)
    yield
    l2s = T_("l2s")
    V.tensor_scalar_mul(l2s, l2p, inv_n)
    yield
    V.scalar_tensor_tensor(out=loss[:, c0:c1], in0=n, scalar=-2.0, in1=l2s,
                           op0=ALU.mult, op1=ALU.add)
    yield


# ---------------------------------------------------------------------------# ---------------------------------------------------------------------------
# host glue
# ---------------------------------------------------------------------------


class Runner:
    """Cached jitted shard_map executor for repeated invocations (timing)."""

    def __init__(self, nc, n_cores=N_CORES):
        import jax
        from jax.experimental.shard_map import shard_map
        from jax.sharding import Mesh, PartitionSpec
        from concourse import bass2jax
        from concourse import mybir as _mybir

        bass2jax.install_neuronx_cc_hook()
        self.nc = nc
        self.n_cores = n_cores
        partition_name = nc.partition_id_tensor.name if nc.partition_id_tensor else None
        in_names, out_names, out_avals, zero_outs = [], [], [], []
        for alloc in nc.m.functions[0].allocations:
            if not isinstance(alloc, _mybir.MemoryLocationSet):
                continue
            name = alloc.memorylocations[0].name
            if alloc.kind == "ExternalInput":
                if name != partition_name:
                    in_names.append(name)
            elif alloc.kind == "ExternalOutput":
                out_names.append(name)
                shape = tuple(alloc.tensor_shape)
                dtype = _mybir.dt.np(alloc.dtype)
                out_avals.append(jax.core.ShapedArray(shape, dtype))
                zero_outs.append(np.zeros(shape, dtype))
        self.in_names = list(in_names)
        self.out_names = out_names
        self.zero_outs = zero_outs
        n_params = len(in_names)
        n_outs = len(out_avals)
        all_in_names = in_names + out_names
        if partition_name is not None:
            all_in_names = all_in_names + [partition_name]

        def _body(*args):
            operands = list(args)
            if partition_name is not None:
                operands.append(bass2jax.partition_id_tensor())
            outs = bass2jax._bass_exec_p.bind(
                *operands,
                out_avals=tuple(out_avals),
                in_names=tuple(all_in_names),
                out_names=tuple(out_names),
                lowering_input_output_aliases=(),
                sim_require_finite=True,
                sim_require_nnan=True,
                nc=nc,
            )
            return tuple(outs)

        devices = jax.devices()[:n_cores]
        mesh = Mesh(np.asarray(devices), ("core",))
        self.mesh = mesh
        in_specs = (PartitionSpec("core"),) * (n_params + n_outs)
        out_specs = (PartitionSpec("core"),) * n_outs
        self.fn = jax.jit(
            shard_map(_body, mesh=mesh, in_specs=in_specs, out_specs=out_specs,
                      check_rep=False),
            keep_unused=True,
        )

    def prep(self, in_maps, device_put=True):
        """in_maps: list of per-core dicts -> concatenated arg list (device-resident)."""
        concat = [
            np.concatenate([np.asarray(in_maps[c][n]) for c in range(self.n_cores)], axis=0)
            for n in self.in_names
        ]
        concat += [
            np.zeros((self.n_cores * z.shape[0], *z.shape[1:]), z.dtype)
            for z in self.zero_outs
        ]
        if device_put:
            import jax
            from jax.sharding import NamedSharding, PartitionSpec

            sh = NamedSharding(self.mesh, PartitionSpec("core"))
            concat = [jax.device_put(a, sh) for a in concat]
            jax.block_until_ready(concat)
        return concat

    def __call__(self, args):
        return self.fn(*args)


_NC_CACHE = {}


def _get_nc(b_core=B_CORE):
    if b_core not in _NC_CACHE:
        _NC_CACHE[b_core] = build_kernel(b_core)
    return _NC_CACHE[b_core]


def _consts():
    sel = np.zeros((128, 128), ml_dtypes.bfloat16)
    sel[:, 127] = 1.0
    idb = np.eye(128, dtype=ml_dtypes.bfloat16)
    idf = np.eye(128, dtype=np.float32)
    return sel, idb, idf


def run_cores(x, y, b_core=B_CORE, n_cores=N_CORES, nc=None):
    """x, y: (n_cores*b_core, 128, 3) float32 -> list of per-core (loss, ssq) grids."""
    if nc is None:
        nc = _get_nc(b_core)
    sel, idb, idf = _consts()
    xs = np.ascontiguousarray(x, dtype=np.float32).reshape(n_cores, b_core, F)
    ys = np.ascontiguousarray(y, dtype=np.float32).reshape(n_cores, b_core, F)
    in_maps = [
        {"x": xs[c], "y": ys[c], "sel": sel, "idb": idb, "idf": idf}
        for c in range(n_cores)
    ]
    res = run_bass_kernel_spmd(nc, in_maps, core_ids=list(range(n_cores)))
    return [(res.results[c]["loss"], res.results[c]["ssq"]) for c in range(n_cores)]


def kernel(x, y):
    """Full-input entry point: x, y (65536, 128, 3) float32 -> scalar float32."""
    grids = run_cores(np.asarray(x), np.asarray(y))
    total = sum(
        g.astype(np.float64).sum() + q.astype(np.float64).sum() for g, q in grids
    )
    return np.float32(total / (B_TOTAL * N_PTS * 3))
